# revision 17
# baseline (speedup 1.0000x reference)
"""JetMoE FFN (top-2 MoE with GLU experts) on 8 Trainium2 NeuronCores.

Strategy (expert parallelism, all on device):
- Each core owns one expert's weights (host slices w_in[e], w_out[e]).
- Each core computes router logits for its 512 home tokens on the Tensor
  engine in f32, then an AllGather shares all 4096 token logits.
- Every core redundantly computes top-2 masks, gates, and block-wise
  exclusive cumsums (via triangular matmuls) giving each selected token a
  slot in a capacity-padded per-(expert, home-block) dispatch layout.
- Each core scatters (token-id, gate) pairs for ITS expert into a compact
  dispatch list via indirect DMA, gathers those x rows, transposes on PE,
  and runs the GLU expert MLP in bf16 (f32 PSUM accumulation).
- Gate-scaled outputs (bf16) return to each token's home core via an
  AllToAll; home cores combine the two expert contributions + bias.
- The aux load-balancing loss is computed on-device from the same logits.

Host only slices/replicates inputs, concatenates the 8 output chunks, and
picks the dispatch capacity (C_PAIR) from the observed routing counts.
"""
import math
import numpy as np

T, D, F, E = 4096, 1024, 2048, 8
F2 = 2 * F
HOME = T // 8
NT = T // 128          # 32 token tiles
TOPK = 2

_cache = {}
_last_c_pair = 160
TRACE = False          # set by test harness to capture NTFF profile
LAST_RESULT = None     # BassKernelResults of the most recent run


def _build(c_pair, sim_nocc=False, skip_t=False, skip_g=False, skip_w=False):
    from contextlib import ExitStack
    import concourse.bass as bass
    import concourse.tile as tile
    import concourse.mybir as mybir
    from concourse import bacc
    from concourse.masks import make_identity

    F32, BF16, I32 = mybir.dt.float32, mybir.dt.bfloat16, mybir.dt.int32
    AX = mybir.AxisListType.X
    OP = mybir.AluOpType
    ACT = mybir.ActivationFunctionType

    CSL = 8 * c_pair       # dispatch slots per core
    TRASH = 4 * c_pair     # scatter target for unselected tokens (per half)
    NSLOT = CSL // 128     # slot tiles
    assert CSL % 128 == 0

    nc = bacc.Bacc("TRN2", target_bir_lowering=False, debug=False, num_devices=8)

    x_full = nc.dram_tensor("x_full", [T, D], F32, kind="ExternalInput")
    x_home = nc.dram_tensor("x_home", [HOME, D], F32, kind="ExternalInput")
    rw_d = nc.dram_tensor("router_w", [D, E], F32, kind="ExternalInput")
    w_in_d = nc.dram_tensor("w_in_e", [D, F2], F32, kind="ExternalInput")
    w_out_d = nc.dram_tensor("w_out_e", [F, D], F32, kind="ExternalInput")
    bias_d = nc.dram_tensor("out_bias", [1, D], F32, kind="ExternalInput")
    eoh_d = nc.dram_tensor("eoh", [128, E], F32, kind="ExternalInput")
    hsel_d = nc.dram_tensor("hsel", [128, E], F32, kind="ExternalInput")

    out_d = nc.dram_tensor("out_chunk", [HOME, D], F32, kind="ExternalOutput")
    aux_d = nc.dram_tensor("aux", [1, 1], F32, kind="ExternalOutput")

    ag_in = nc.dram_tensor("ag_in", [HOME, E], F32)
    lg_full = nc.dram_tensor("lg_full", [T, E], F32, addr_space="Shared")
    HSL = CSL // 2
    pk_d = nc.dram_tensor("pk_d", [HSL + 128, 2], F32)    # blocks 0-3
    pk_d2 = nc.dram_tensor("pk_d2", [HSL + 128, 2], F32)  # blocks 4-7
    send_d = nc.dram_tensor("send_d", [CSL, D], BF16)
    recv_d = nc.dram_tensor("recv_d", [CSL, D], BF16)

    RG = [list(range(8))]

    with tile.TileContext(nc) as tc, ExitStack() as ctx:
        keep = ctx.enter_context(tc.tile_pool(name="keep", bufs=1))
        wpool = ctx.enter_context(tc.tile_pool(name="wpool", bufs=1))

        # ---------------- resident weights (bf16, cast during DMA) --------
        w_in_sb = wpool.tile([128, D // 128, F2], BF16)
        for kd in range(0 if skip_w else D // 128):
            nc.gpsimd.dma_start(w_in_sb[:, kd, :], w_in_d[kd * 128:(kd + 1) * 128, :])
        w_out_sb = wpool.tile([128, F // 128, D], BF16)
        for kf in range(0 if skip_w else F // 128):
            nc.gpsimd.dma_start(w_out_sb[:, kf, :], w_out_d[kf * 128:(kf + 1) * 128, :])

        ident = keep.tile([128, 128], F32)
        make_identity(nc, ident[:])
        ident_bf = keep.tile([128, 128], BF16)
        nc.vector.tensor_copy(ident_bf[:], ident[:])
        ones128 = keep.tile([128, 1], F32)
        nc.vector.memset(ones128[:], 1.0)

        eoh = keep.tile([128, E], F32)
        nc.sync.dma_start(eoh[:], eoh_d[:])
        hsel = keep.tile([128, E], F32)
        nc.sync.dma_start(hsel[:], hsel_d[:])

        # persistent routing results
        off_i32 = keep.tile([128, NT], I32)       # dispatch slot per token (my expert)
        pk_vals = keep.tile([128, NT, 2], F32)    # (token-id, gate) scatter payload
        oh1 = keep.tile([128, 4], I32)            # home-token recv rows (expert 1)
        oh2 = keep.tile([128, 4], I32)
        accS = keep.tile([128, E], F32)
        accP = keep.tile([128, E], F32)
        nc.vector.memset(accS[:], 0.0)
        nc.vector.memset(accP[:], 0.0)

        # ---------------- phase R: home router + AllGather ----------------
        with tc.tile_pool(name="rt_sb", bufs=3) as rsb, \
             tc.tile_pool(name="rt_ps", bufs=2, space="PSUM") as rps:
            rw_sb = rsb.tile([128, D // 128, E], F32, tag="rw")
            nc.sync.dma_start(rw_sb[:], rw_d.rearrange("(k p) e -> p k e", p=128))
            lg_home = rsb.tile([128, 4, E], F32, tag="lgh")
            for kt in range(4):
                xh = rsb.tile([128, D], F32, tag="xh")
                nc.sync.dma_start(xh[:], x_home[kt * 128:(kt + 1) * 128, :])
                xt = rsb.tile([128, D // 128, 128], F32, tag="xt")
                for kd in range(D // 128):
                    pt = rps.tile([128, 128], F32, tag="pt", space="PSUM")
                    nc.tensor.transpose(out=pt[:], in_=xh[:, kd * 128:(kd + 1) * 128],
                                        identity=ident[:])
                    nc.vector.tensor_copy(xt[:, kd, :], pt[:])
                lp = rps.tile([128, E], F32, tag="lp", space="PSUM")
                for kd in range(D // 128):
                    nc.tensor.matmul(out=lp[:], lhsT=xt[:, kd, :], rhs=rw_sb[:, kd, :],
                                     start=(kd == 0), stop=(kd == D // 128 - 1))
                nc.vector.tensor_copy(lg_home[:, kt, :], lp[:])
            nc.sync.dma_start(ag_in.rearrange("(k p) e -> p k e", p=128), lg_home[:])
            if sim_nocc:
                for _j in range(8):
                    nc.sync.dma_start(lg_full[_j * HOME:(_j + 1) * HOME, :], ag_in[:])
            else:
                nc.gpsimd.collective_compute(
                    "AllGather", OP.bypass, replica_groups=RG,
                    ins=[ag_in[:]], outs=[lg_full[:]])

        # ---------------- phase T: top-2, gates, slots (batched) ----------
        with tc.tile_pool(name="tt_sb", bufs=2) as tsb, \
             tc.tile_pool(name="tt_ps", bufs=2, space="PSUM") as tps:
            lgt = tsb.tile([128, NT, E], F32, tag="lgt")
            nc.sync.dma_start(lgt[:], lg_full.rearrange("(i p) e -> p i e", p=128))

            Lstrict = tsb.tile([128, 128], F32, tag="Ls")
            nc.gpsimd.memset(Lstrict[:], 0.0)
            # keep 0 where (p - f) >= 0, fill 1 where p < f  -> L[p,f]=1 iff p<f
            nc.gpsimd.affine_select(out=Lstrict[:], in_=Lstrict[:],
                                    pattern=[[-1, 128]], base=0,
                                    channel_multiplier=1,
                                    compare_op=OP.is_ge, fill=1.0)
            onesm = tsb.tile([128, 128], F32, tag="om")
            nc.vector.memset(onesm[:], 1.0)
            erow_f = tsb.tile([128, E], F32, tag="er")
            erow_i = tsb.tile([128, E], I32, tag="eri")
            nc.gpsimd.iota(erow_i[:], pattern=[[c_pair, E]], base=0,
                           channel_multiplier=0)
            nc.vector.tensor_copy(erow_f[:], erow_i[:])
            # token ids as exact f32 (payload col 0): id = p + 128*i
            iota_tmp = tsb.tile([128, NT], I32, tag="iot")
            nc.gpsimd.iota(iota_tmp[:], pattern=[[128, NT]], base=0,
                           channel_multiplier=1)
            nc.vector.tensor_copy(pk_vals[:, :, 0], iota_tmp[:])
            # per-column block base b*c_pair (i = b*4 + m)
            bvec_i = tsb.tile([128, NT], I32, tag="bvi")
            nc.gpsimd.iota(bvec_i[:], pattern=[[0, 2], [c_pair, 4], [0, 4]], base=0,
                           channel_multiplier=0)
            bvec_f = tsb.tile([128, NT], F32, tag="bvf")
            nc.vector.tensor_copy(bvec_f[:], bvec_i[:])

            # ---- batched top-2 + masks + gates over all 32 tiles ----
            m1a = tsb.tile([128, NT, 1], F32, tag="m1a")
            nc.vector.reduce_max(m1a[:, :, 0], lgt[:], axis=AX)
            mask1 = tsb.tile([128, NT, E], F32, tag="km1")
            nc.vector.tensor_tensor(mask1[:], lgt[:],
                                    m1a[:].to_broadcast([128, NT, E]),
                                    op=OP.is_equal)
            big = tsb.tile([128, NT, E], F32, tag="big")
            nc.vector.tensor_scalar(big[:], mask1[:], 1e30, None, op0=OP.mult)
            nc.vector.tensor_sub(big[:], lgt[:], big[:])
            m2a = tsb.tile([128, NT, 1], F32, tag="m2a")
            nc.vector.reduce_max(m2a[:, :, 0], big[:], axis=AX)
            mask2 = tsb.tile([128, NT, E], F32, tag="km2")
            nc.vector.tensor_tensor(mask2[:], lgt[:],
                                    m2a[:].to_broadcast([128, NT, E]),
                                    op=OP.is_equal)
            S_all = tsb.tile([128, NT, E], F32, tag="Sa")
            nc.vector.tensor_add(S_all[:], mask1[:], mask2[:])
            d_all = tsb.tile([128, NT], F32, tag="da")
            nc.vector.tensor_sub(d_all[:], m1a[:, :, 0], m2a[:, :, 0])
            g1a = tsb.tile([128, NT, 1], F32, tag="g1a")
            nc.scalar.activation(g1a[:, :, 0], d_all[:], ACT.Sigmoid)
            W_all = tsb.tile([128, NT, E], F32, tag="Wa")
            t0_ = tsb.tile([128, NT, E], F32, tag="t0")
            nc.vector.tensor_tensor(t0_[:], mask1[:],
                                    g1a[:].to_broadcast([128, NT, E]), op=OP.mult)
            t1_ = tsb.tile([128, NT, E], F32, tag="t1")
            nc.vector.tensor_tensor(t1_[:], mask2[:],
                                    g1a[:].to_broadcast([128, NT, E]), op=OP.mult)
            nc.vector.tensor_sub(t1_[:], mask2[:], t1_[:])
            nc.vector.tensor_add(W_all[:], t0_[:], t1_[:])
            # ---- aux accumulators (logits are small: exp without max-shift)
            ex = tsb.tile([128, NT, E], F32, tag="exa")
            nc.scalar.activation(ex[:], lgt[:], ACT.Exp)
            se = tsb.tile([128, NT], F32, tag="sea")
            nc.vector.reduce_sum(se[:], ex[:], axis=AX)
            rs = tsb.tile([128, NT, 1], F32, tag="rsa")
            nc.vector.reciprocal(rs[:, :, 0], se[:])
            nc.vector.tensor_tensor(ex[:], ex[:],
                                    rs[:].to_broadcast([128, NT, E]), op=OP.mult)
            nc.vector.reduce_sum(accP[:], ex[:].rearrange("p i e -> p e i"), axis=AX)
            nc.vector.reduce_sum(accS[:], S_all[:].rearrange("p i e -> p e i"), axis=AX)
            # ---- block-local exclusive cumsum (PE) ----
            r_all = tsb.tile([128, NT, E], F32, tag="ra")
            for i in range(0 if skip_t else NT):
                b = i // 4
                rp = tps.tile([128, E], F32, tag="rp", space="PSUM")
                first = True
                for ii in range(4 * b, i):
                    nc.tensor.matmul(out=rp[:], lhsT=onesm[:], rhs=S_all[:, ii, :],
                                     start=first, stop=False)
                    first = False
                nc.tensor.matmul(out=rp[:], lhsT=Lstrict[:], rhs=S_all[:, i, :],
                                 start=first, stop=True)
                nc.vector.tensor_copy(r_all[:, i, :], rp[:])
            # ---- batched column selects for my expert ----
            eohb = eoh[:][:, None, :].to_broadcast([128, NT, E])
            sel = tsb.tile([128, NT, E], F32, tag="sel")
            rsel = tsb.tile([128, NT], F32, tag="rsl")
            nc.vector.tensor_tensor(sel[:], r_all[:], eohb, op=OP.mult)
            nc.vector.reduce_sum(rsel[:], sel[:], axis=AX)
            ssel = tsb.tile([128, NT], F32, tag="ssl")
            nc.vector.tensor_tensor(sel[:], S_all[:], eohb, op=OP.mult)
            nc.vector.reduce_sum(ssel[:], sel[:], axis=AX)
            nc.vector.tensor_tensor(sel[:], W_all[:], eohb, op=OP.mult)
            nc.vector.reduce_sum(pk_vals[:, :, 1], sel[:], axis=AX)
            # off = ssel*(rsel + b*c_pair - TRASH) + TRASH
            offa = tsb.tile([128, NT], F32, tag="ofa")
            nc.vector.tensor_add(offa[:], rsel[:], bvec_f[:])
            nc.vector.tensor_scalar(offa[:], offa[:], float(-TRASH), None, op0=OP.add)
            nc.vector.tensor_tensor(offa[:], offa[:], ssel[:], op=OP.mult)
            nc.vector.tensor_scalar(offa[:], offa[:], float(TRASH), None, op0=OP.add)
            nc.vector.tensor_copy(off_i32[:], offa[:])
            # ---- batched home-side recv rows ----
            era = tsb.tile([128, NT, E], F32, tag="era")
            nc.vector.tensor_add(era[:], r_all[:],
                                 erow_f[:][:, None, :].to_broadcast([128, NT, E]))
            hselb = hsel[:][:, None, :].to_broadcast([128, 4, E])
            for msk, oh in ((mask1, oh1), (mask2, oh2)):
                nc.vector.tensor_tensor(sel[:], msk[:], era[:], op=OP.mult)
                offh = tsb.tile([128, NT], F32, tag="ofh")
                nc.vector.reduce_sum(offh[:], sel[:], axis=AX)
                hsl = tsb.tile([128, 4], F32, tag="hsl")
                hmul = tsb.tile([128, 4, E], F32, tag="hmu")
                nc.vector.tensor_tensor(hmul[:],
                                        offh[:].rearrange("p (b m) -> p m b", m=4),
                                        hselb, op=OP.mult)
                nc.vector.reduce_sum(hsl[:], hmul[:], axis=AX)
                nc.vector.tensor_copy(oh[:], hsl[:])
            # ---- aux loss ----
            sp = tps.tile([E, 1], F32, tag="sp", space="PSUM")
            nc.tensor.matmul(out=sp[:], lhsT=accS[:], rhs=ones128[:],
                             start=True, stop=True)
            ssum = tsb.tile([E, 1], F32, tag="ssum")
            nc.vector.tensor_copy(ssum[:], sp[:])
            pp = tps.tile([E, 1], F32, tag="pp", space="PSUM")
            nc.tensor.matmul(out=pp[:], lhsT=accP[:], rhs=ones128[:],
                             start=True, stop=True)
            fp = tsb.tile([E, 1], F32, tag="fp")
            nc.vector.tensor_tensor(fp[:], ssum[:], pp[:], op=OP.mult)
            ap2 = tps.tile([1, 1], F32, tag="ap2", space="PSUM")
            nc.tensor.matmul(out=ap2[:], lhsT=fp[:], rhs=ones128[:E, :],
                             start=True, stop=True)
            auxs = tsb.tile([1, 1], F32, tag="auxs")
            nc.scalar.mul(auxs[:], ap2[:], float(E) / (float(T) * T * TOPK))
            nc.sync.dma_start(aux_d[:], auxs[:])
            # ---- prefill + scatter dispatch pairs ----
            zpk = tsb.tile([128, (HSL + 128) // 128, 2], F32, tag="zpk")
            nc.vector.memset(zpk[:], 0.0)
            nc.sync.dma_start(pk_d.rearrange("(k p) t -> p k t", p=128), zpk[:])
            nc.sync.dma_start(pk_d2.rearrange("(k p) t -> p k t", p=128), zpk[:])
            for i in range(NT):
                tgt = pk_d if i < 16 else pk_d2
                nc.gpsimd.indirect_dma_start(
                    out=tgt[:],
                    out_offset=bass.IndirectOffsetOnAxis(ap=off_i32[:, i:i + 1], axis=0),
                    in_=pk_vals[:, i, :], in_offset=None)

        # ---------------- phase G: gather + expert MLP --------------------
        gsb = ctx.enter_context(tc.tile_pool(name="g_sb", bufs=2))
        apool = ctx.enter_context(tc.tile_pool(name="g_a", bufs=1))
        yapool = ctx.enter_context(tc.tile_pool(name="g_ya", bufs=2))
        sgpool = ctx.enter_context(tc.tile_pool(name="g_sg", bufs=1))
        psg = ctx.enter_context(tc.tile_pool(name="ps_g", bufs=2, space="PSUM"))
        psv = ctx.enter_context(tc.tile_pool(name="ps_v", bufs=2, space="PSUM"))
        psy = ctx.enter_context(tc.tile_pool(name="ps_y", bufs=2, space="PSUM"))
        pst = ctx.enter_context(tc.tile_pool(name="ps_t", bufs=2, space="PSUM"))

        gidx = keep.tile([128, NSLOT], I32)
        gate_sb = keep.tile([128, NSLOT], F32)
        HT = HSL // 128
        gf = gsb.tile([128, NSLOT], F32, tag="gf")
        for half, pkt in ((0, pk_d), (1, pk_d2)):
            nc.sync.dma_start(gf[:, half * HT:(half + 1) * HT],
                              pkt.rearrange("(k p) t -> p k t", p=128)[:, :HT, 0])
            nc.sync.dma_start(gate_sb[:, half * HT:(half + 1) * HT],
                              pkt.rearrange("(k p) t -> p k t", p=128)[:, :HT, 1])
        nc.vector.tensor_copy(gidx[:], gf[:])

        # first chunk stays inside half A so compute starts as soon as the
        # first 16 scatters land; later chunks may span the half boundary
        # (half B's scatters complete during chunk-0 compute) — fewer chunks
        # means fewer fixed-count PE instruction groups
        chunks = [(0, min(4, HT))]
        c0 = chunks[0][1]
        while c0 < NSLOT:
            w = min(4, NSLOT - c0)
            chunks.append((c0, w))
            c0 += w

        for (c0, w) in ([] if skip_g else chunks):
            cw = 128 * w
            xg = gsb.tile([128, 4, D], F32, tag="xg")
            for s in range(w):
                nc.gpsimd.indirect_dma_start(
                    out=xg[:, s, :], out_offset=None, in_=x_full[:],
                    in_offset=bass.IndirectOffsetOnAxis(
                        ap=gidx[:, c0 + s:c0 + s + 1], axis=0))
            xT = gsb.tile([128, D // 128, 512], BF16, tag="xT")
            for s in range(w):
                for kd in range(D // 128):
                    pt = pst.tile([128, 128], F32, tag="pt", space="PSUM")
                    nc.tensor.transpose(out=pt[:], in_=xg[:, s, kd * 128:(kd + 1) * 128],
                                        identity=ident[:])
                    nc.vector.tensor_copy(xT[:, kd, s * 128:(s + 1) * 128], pt[:])
            aT = apool.tile([128, F // 128, 512], BF16, tag="aT")
            for mi in range(F // 128):
                pg = psg.tile([128, cw], F32, tag="pg", space="PSUM")
                pv = psv.tile([128, cw], F32, tag="pv", space="PSUM")
                for kd in range(D // 128):
                    nc.tensor.matmul(out=pg[:], rhs=xT[:, kd, :cw],
                                     lhsT=w_in_sb[:, kd, mi * 128:(mi + 1) * 128],
                                     start=(kd == 0), stop=(kd == D // 128 - 1))
                for kd in range(D // 128):
                    nc.tensor.matmul(out=pv[:], rhs=xT[:, kd, :cw],
                                     lhsT=w_in_sb[:, kd, F + mi * 128:F + (mi + 1) * 128],
                                     start=(kd == 0), stop=(kd == D // 128 - 1))
                sg = sgpool.tile([128, 512], F32, tag="sg")
                nc.scalar.activation(sg[:, :cw], pg[:], ACT.Silu)
                nc.vector.tensor_tensor(aT[:, mi, :cw], sg[:, :cw], pv[:], op=OP.mult)
            yt = gsb.tile([128, D // 128, 512], BF16, tag="yt")
            for di in range(D // 128):
                py = psy.tile([128, cw], F32, tag="py", space="PSUM")
                for mi in range(F // 128):
                    nc.tensor.matmul(out=py[:], rhs=aT[:, mi, :cw],
                                     lhsT=w_out_sb[:, mi, di * 128:(di + 1) * 128],
                                     start=(mi == 0), stop=(mi == F // 128 - 1))
                nc.vector.tensor_copy(yt[:, di, :cw], py[:])
            for s in range(w):
                ya = yapool.tile([128, D], BF16, tag="ya")
                for di in range(D // 128):
                    pt2 = pst.tile([128, 128], BF16, tag="pt", space="PSUM")
                    nc.tensor.transpose(out=pt2[:], in_=yt[:, di, s * 128:(s + 1) * 128],
                                        identity=ident_bf[:])
                    nc.vector.tensor_scalar(ya[:, di * 128:(di + 1) * 128], pt2[:],
                                            gate_sb[:, c0 + s:c0 + s + 1], None,
                                            op0=OP.mult)
                nc.sync.dma_start(send_d[(c0 + s) * 128:(c0 + s + 1) * 128, :], ya[:])

        # ---------------- phase C: return A2A + home combine --------------
        if sim_nocc:
            nc.sync.dma_start(recv_d[:], send_d[:])
        else:
            nc.gpsimd.collective_compute(
                "AllToAll", mybir.AluOpType.bypass, replica_groups=RG,
                ins=[send_d[:]], outs=[recv_d[:]])
        bias_sb = keep.tile([1, D], F32)
        nc.sync.dma_start(bias_sb[:], bias_d[:])
        bias_bc = keep.tile([128, D], F32)
        nc.gpsimd.partition_broadcast(bias_bc[:], bias_sb[:])
        for m in range(4):
            ga = yapool.tile([128, D], BF16, tag="ga")
            nc.gpsimd.indirect_dma_start(
                out=ga[:], out_offset=None, in_=recv_d[:],
                in_offset=bass.IndirectOffsetOnAxis(ap=oh1[:, m:m + 1], axis=0))
            gb = yapool.tile([128, D], BF16, tag="gb")
            nc.gpsimd.indirect_dma_start(
                out=gb[:], out_offset=None, in_=recv_d[:],
                in_offset=bass.IndirectOffsetOnAxis(ap=oh2[:, m:m + 1], axis=0))
            s1 = gsb.tile([128, D], F32, tag="s1")
            nc.vector.tensor_add(s1[:], ga[:], gb[:])
            nc.vector.tensor_add(s1[:], s1[:], bias_bc[:])
            nc.sync.dma_start(out_d[m * 128:(m + 1) * 128, :], s1[:])

    nc.compile()
    return nc


def kernel(x, router_w, w_in, w_out, out_bias, top_k):
    from concourse.bass_utils import run_bass_kernel_spmd

    x = np.ascontiguousarray(np.asarray(x, dtype=np.float32))
    router_w = np.ascontiguousarray(np.asarray(router_w, dtype=np.float32))
    w_in = np.ascontiguousarray(np.asarray(w_in, dtype=np.float32))
    w_out = np.ascontiguousarray(np.asarray(w_out, dtype=np.float32))
    out_bias = np.ascontiguousarray(np.asarray(out_bias, dtype=np.float32))
    assert int(top_k) == 2
    B, S, _ = x.shape
    xt = x.reshape(T, D)

    # host: pick dispatch capacity from observed per-(expert, home) counts
    logits = xt @ router_w
    top2 = np.argpartition(-logits, 2, axis=1)[:, :2]
    sel = np.zeros((T, E), bool)
    sel[np.arange(T)[:, None], top2] = True
    pair_max = int(sel.reshape(8, HOME, E).sum(1).max())
    c_pair = max(32, int(math.ceil(pair_max / 16.0)) * 16)

    global _last_c_pair
    _last_c_pair = c_pair
    if c_pair not in _cache:
        _cache[c_pair] = _build(c_pair)
    nc = _cache[c_pair]

    eye = np.eye(E, dtype=np.float32)
    in_maps = []
    for c in range(8):
        in_maps.append({
            "x_full": xt,
            "x_home": xt[c * HOME:(c + 1) * HOME],
            "router_w": router_w,
            "w_in_e": w_in[c],
            "w_out_e": w_out[c],
            "out_bias": out_bias.reshape(1, D),
            "eoh": np.repeat(eye[c][None, :], 128, 0),
            "hsel": np.repeat(eye[c][None, :], 128, 0),
        })
    global LAST_RESULT
    if TRACE:
        try:
            res = run_bass_kernel_spmd(nc, in_maps, list(range(8)), trace=True,
                                       trace_cores=list(range(8)),
                                       stitch_traces=True)
        except Exception:
            res = run_bass_kernel_spmd(nc, in_maps, list(range(8)))
    else:
        res = run_bass_kernel_spmd(nc, in_maps, list(range(8)))
    LAST_RESULT = res
    out = np.concatenate([res.results[c]["out_chunk"] for c in range(8)], axis=0)
    out = out.reshape(B, S, D).astype(np.float32)
    aux = np.float32(res.results[0]["aux"][0, 0])
    return out, aux
